# revision 7
# baseline (speedup 1.0000x reference)
"""Trainium2 Bass kernel for ContrastiveMSELoss.

Reference computes, over all N^2 pairs (diagonal masked to 0):
    mse_ij  = (|x_i|^2 + |x_j|^2 - 2 x_i.x_j) / D
    sign_ij = +1 if class_i == class_j else -1
    loss    = mean_ij(sign_ij * mse_ij) + BETA

Using sum_{i,j in c} x_i.x_j = |M_c|^2 with M_c = sum_{i in c} x_i, the
loss collapses to class-bucketed first/second moments (O(N*D) work,
memory-bound -- no N x N gram matrix needed):

    T_same = sum_c (2 n_c SQ_c - 2 |M_c|^2) / D      (diag terms are 0)
    T_all  = (2 N SQ - 2 |M|^2) / D
    loss   = (2 T_same - T_all) / N^2 + BETA

Sharding: rows are split across 8 cores (1024 rows each).  Per core the
shard is mapped row r = p*8 + k onto partition p, sub-chunk k, so the HBM
image of the SBUF tile is fully contiguous per partition (8 KB
descriptors -> near line-rate DMA, 4 column-block transfers pipelined
with compute).  A one-hot-classes matmul accumulates per-class sums of
[X | X^2] in PSUM; even sub-chunks accumulate into PSUM partitions 0:40,
odd into 64:104 so consecutive matmuls land in different PE column
groups and overlap.  Host combines the per-core [80, 512] partials.
"""

import numpy as np

import concourse.bacc as bacc
import concourse.bass as bass
import concourse.tile as tile
from concourse import mybir
from concourse.bass_utils import run_bass_kernel_spmd

N, D = 8192, 256
N_CORES = 8
ROWS = N // N_CORES          # 1024 rows per core
P = 128                      # partitions
K = ROWS // P                # 8 sub-rows per partition (row = p*K + k)
NCLS = 40
BETA = 1.0
NBLK = 4                     # column-block DMAs; block b covers k = 2b, 2b+1
BLKW = (K // NBLK) * D       # 512 f32 elements per partition per block

_CACHE = {}


def _bcast(ap, pos, count):
    """Insert a zero-stride dim of size `count` at free-dim position `pos`."""
    pattern = [list(p) for p in ap.ap]
    pattern.insert(pos, [0, count])
    return bass.AP(tensor=ap.tensor, offset=ap.offset, ap=pattern)


def _build_bass():
    nc = bacc.Bacc(
        "TRN2",
        target_bir_lowering=False,
        debug=False,
        enable_asserts=False,
        num_devices=N_CORES,
    )
    # x shard viewed as [128, 8, 256]: partition p = rows p*8 .. p*8+7
    x = nc.dram_tensor("x", [P, K, D], mybir.dt.float32, kind="ExternalInput")
    # combo[p, :NCLS] = iota row 0..39; combo[p, NCLS + k] = class of row p*8+k
    combo = nc.dram_tensor(
        "combo", [P, NCLS + K], mybir.dt.float32, kind="ExternalInput"
    )
    # stats rows 0:40 = even-k chain, 40:80 = odd-k chain;
    # cols 0:256 per-class sums of x, 256:512 per-class sums of x^2
    stats = nc.dram_tensor(
        "stats", [2 * NCLS, 2 * D], mybir.dt.float32, kind="ExternalOutput"
    )

    with tile.TileContext(nc) as tc:
        with (
            tc.tile_pool(name="work", bufs=1) as work,
            tc.tile_pool(name="psum", bufs=1, space="PSUM") as psum_pool,
        ):
            xf = work.tile([P, K, D], mybir.dt.float32, tag="xf")
            combo_sb = work.tile([P, NCLS + K], mybir.dt.float32, tag="combo_sb")
            xb = work.tile([P, K, 2 * D], mybir.dt.bfloat16, tag="xb")
            oh = work.tile([P, K, NCLS], mybir.dt.bfloat16, tag="oh")
            acc = psum_pool.tile([P, 2 * D], mybir.dt.float32, tag="acc")
            out_sb = work.tile([P, 2 * D], mybir.dt.float32, tag="out_sb")

            # Input DMAs: 2 KB contiguous per partition per block.  Scalar
            # (ACT hwdge ring) takes the small combo + block 1 so the Sync
            # ring isn't the only one streaming; Sync takes blocks 0/2/3.
            nc.scalar.dma_start(out=combo_sb, in_=combo[:, :])
            nc.sync.dma_start(out=xf[:, 0:2, :], in_=x[:, 0:2, :])
            nc.scalar.dma_start(out=xf[:, 2:4, :], in_=x[:, 2:4, :])
            nc.sync.dma_start(out=xf[:, 4:6, :], in_=x[:, 4:6, :])
            nc.sync.dma_start(out=xf[:, 6:8, :], in_=x[:, 6:8, :])

            iota_sb = combo_sb[:, :NCLS]
            cls_sb = combo_sb[:, NCLS:]

            # one-hot: oh[p, k, c] = (cls[p, k] == c)
            nc.vector.tensor_tensor(
                out=oh[:, :, :],
                in0=_bcast(cls_sb, 2, NCLS),
                in1=_bcast(iota_sb, 1, K),
                op=mybir.AluOpType.is_equal,
            )

            for b in range(NBLK):
                ks = (2 * b, 2 * b + 1)
                # cast X -> bf16 (DVE, one strided op per block)
                nc.vector.tensor_copy(
                    xb[:, ks[0] : ks[0] + 2, :D], xf[:, ks[0] : ks[0] + 2, :]
                )
                # X^2 -> bf16: DVE (bf16 2x) for blocks 0/2, ACT (from f32)
                # for blocks 1/3 so neither engine paces the chain alone
                if b % 2 == 0:
                    nc.vector.tensor_mul(
                        xb[:, ks[0] : ks[0] + 2, D:],
                        xb[:, ks[0] : ks[0] + 2, :D],
                        xb[:, ks[0] : ks[0] + 2, :D],
                    )
                else:
                    nc.scalar.activation(
                        out=xb[:, ks[0] : ks[0] + 2, D:],
                        in_=xf[:, ks[0] : ks[0] + 2, :],
                        func=mybir.ActivationFunctionType.Square,
                    )
                for k in ks:
                    lo = 0 if k % 2 == 0 else 64
                    nc.tensor.matmul(
                        acc[lo : lo + NCLS, :],
                        oh[:, k, :],
                        xb[:, k, :],
                        start=(k < 2),
                        stop=(k >= K - 2),
                        skip_group_check=True,
                    )

            # fold both chains' PSUM slices to SBUF on two engines in
            # parallel (compute dst partition base must be 32-aligned, so
            # chain B stays at partition 64) and DMA both out concurrently
            nc.vector.tensor_copy(out_sb[:NCLS, :], acc[:NCLS, :])
            nc.scalar.copy(out_sb[64 : 64 + NCLS, :], acc[64 : 64 + NCLS, :])
            nc.sync.dma_start(out=stats[:NCLS, :], in_=out_sb[:NCLS, :])
            nc.scalar.dma_start(out=stats[NCLS:, :], in_=out_sb[64 : 64 + NCLS, :])

    return nc


def _get_nc():
    if "nc" not in _CACHE:
        nc = _build_bass()
        nc.finalize()
        _CACHE["nc"] = nc
    return _CACHE["nc"]


_IOTA = np.broadcast_to(np.arange(NCLS, dtype=np.float32), (P, NCLS))


def run_device(output, classes, **spmd_kwargs):
    """Run the per-core Bass kernel; returns (list of per-core stats, results)."""
    x = np.ascontiguousarray(np.asarray(output), dtype=np.float32)
    cls_f = np.asarray(classes).astype(np.float32)
    in_maps = []
    for s in range(N_CORES):
        xs = x[s * ROWS : (s + 1) * ROWS].reshape(P, K, D)
        cs = cls_f[s * ROWS : (s + 1) * ROWS].reshape(P, K)
        combo = np.concatenate([_IOTA, cs], axis=1)
        in_maps.append({"x": np.ascontiguousarray(xs), "combo": np.ascontiguousarray(combo)})
    res = run_bass_kernel_spmd(
        _get_nc(), in_maps, core_ids=list(range(N_CORES)), **spmd_kwargs
    )
    stats = [res.results[s]["stats"] for s in range(N_CORES)]
    return stats, res


def _combine(stats, classes):
    """Combine per-core partial class stats into the scalar loss (float64)."""
    tot = np.sum(np.asarray(stats, dtype=np.float64), axis=0)  # [80, 512]
    tot = tot[:NCLS] + tot[NCLS:]                              # [40, 512]
    M_c = tot[:, :D]                                           # class sums
    SQ_c = tot[:, D:].sum(axis=1)                              # class |x|^2 sums
    n_c = np.bincount(np.asarray(classes).astype(np.int64), minlength=NCLS).astype(
        np.float64
    )
    SQ = SQ_c.sum()
    M = M_c.sum(axis=0)
    T_same = (2.0 * (n_c * SQ_c).sum() - 2.0 * (M_c * M_c).sum()) / D
    T_all = (2.0 * N * SQ - 2.0 * (M @ M)) / D
    loss = (2.0 * T_same - T_all) / (float(N) * float(N)) + BETA
    return np.float32(loss)


def kernel(output, classes):
    stats, _ = run_device(output, classes)
    return _combine(stats, classes)


# revision 8
# speedup vs baseline: 1.0614x; 1.0614x over previous
"""Trainium2 Bass kernel for ContrastiveMSELoss.

Reference computes, over all N^2 pairs (diagonal masked to 0):
    mse_ij  = (|x_i|^2 + |x_j|^2 - 2 x_i.x_j) / D
    sign_ij = +1 if class_i == class_j else -1
    loss    = mean_ij(sign_ij * mse_ij) + BETA

Using sum_{i,j in c} x_i.x_j = |M_c|^2 with M_c = sum_{i in c} x_i, the
loss collapses to class-bucketed first/second moments (O(N*D) work,
memory-bound -- no N x N gram matrix needed):

    T_same = sum_c (2 n_c SQ_c - 2 |M_c|^2) / D      (diag terms are 0)
    T_all  = (2 N SQ - 2 |M|^2) / D
    loss   = (2 T_same - T_all) / N^2 + BETA

Sharding: rows are split across 8 cores (1024 rows each).  Per core the
shard maps row r = p*8 + k onto partition p, sub-chunk k, so each DMA
block is contiguous per partition (2 KB descriptors, per-SDMA-engine
line rate).  The four x blocks alternate between the two HWDGE rings
(sync/scalar) so early blocks' completions are not delayed behind later
blocks queued on the same ring; the tiny class tensor rides the gpsimd
SWDGE ring.  A one-hot-classes matmul accumulates per-class sums of
[X | X^2] in PSUM; even sub-chunks go to PSUM partitions 0:40, odd to
64:104, so consecutive matmuls land in different PE column groups and
run concurrently.  Host combines the per-core [104, 512] partials
(rows 40:64 are don't-care padding).
"""

import numpy as np

import concourse.bacc as bacc
import concourse.bass as bass
import concourse.tile as tile
from concourse import mybir
from concourse.bass_utils import run_bass_kernel_spmd

N, D = 8192, 256
N_CORES = 8
ROWS = N // N_CORES          # 1024 rows per core
P = 128                      # partitions
K = ROWS // P                # 8 sub-rows per partition (row = p*K + k)
NCLS = 40
BETA = 1.0
OUTP = 104                   # output partitions: rows 0:40 + garbage + 64:104

_CACHE = {}


def _bcast(ap, pos, count):
    """Insert a zero-stride dim of size `count` at free-dim position `pos`."""
    pattern = [list(p) for p in ap.ap]
    pattern.insert(pos, [0, count])
    return bass.AP(tensor=ap.tensor, offset=ap.offset, ap=pattern)


def _build_bass():
    nc = bacc.Bacc(
        "TRN2",
        target_bir_lowering=False,
        debug=False,
        enable_asserts=False,
        num_devices=N_CORES,
    )
    # x shard viewed as [128, 8, 256]: partition p = rows p*8 .. p*8+7
    x = nc.dram_tensor("x", [P, K, D], mybir.dt.float32, kind="ExternalInput")
    # combo[p, :NCLS] = iota row 0..39; combo[p, NCLS + k] = class of row p*8+k
    combo = nc.dram_tensor(
        "combo", [P, NCLS + K], mybir.dt.float32, kind="ExternalInput"
    )
    # stats rows 0:40 = even-k chain, 64:104 = odd-k chain, 40:64 garbage;
    # cols 0:256 per-class sums of x, 256:512 per-class sums of x^2
    stats = nc.dram_tensor(
        "stats", [OUTP, 2 * D], mybir.dt.bfloat16, kind="ExternalOutput"
    )

    with tile.TileContext(nc) as tc:
        with (
            tc.tile_pool(name="work", bufs=1) as work,
            tc.tile_pool(name="psum", bufs=1, space="PSUM") as psum_pool,
        ):
            xf = work.tile([P, K, D], mybir.dt.float32, tag="xf")
            combo_sb = work.tile([P, NCLS + K], mybir.dt.float32, tag="combo_sb")
            xb = work.tile([P, K, 2 * D], mybir.dt.bfloat16, tag="xb")
            oh = work.tile([P, K, NCLS], mybir.dt.bfloat16, tag="oh")
            acc = psum_pool.tile([P, 2 * D], mybir.dt.float32, tag="acc")
            out_sb = work.tile([P, 2 * D], mybir.dt.bfloat16, tag="out_sb")

            # Input DMAs: blocks alternate HWDGE rings so each ring holds at
            # most two transfers and early completions aren't held hostage
            # by later blocks on the same ring.  combo rides gpsimd SWDGE.
            nc.gpsimd.dma_start(out=combo_sb, in_=combo[:, :])
            nc.sync.dma_start(out=xf[:, 0:2, :], in_=x[:, 0:2, :])
            nc.scalar.dma_start(out=xf[:, 2:4, :], in_=x[:, 2:4, :])
            nc.sync.dma_start(out=xf[:, 4:6, :], in_=x[:, 4:6, :])
            nc.scalar.dma_start(out=xf[:, 6:8, :], in_=x[:, 6:8, :])

            iota_sb = combo_sb[:, :NCLS]
            cls_sb = combo_sb[:, NCLS:]

            # one-hot: oh[p, k, c] = (cls[p, k] == c)
            nc.vector.tensor_tensor(
                out=oh[:, :, :],
                in0=_bcast(cls_sb, 2, NCLS),
                in1=_bcast(iota_sb, 1, K),
                op=mybir.AluOpType.is_equal,
            )

            for b in range(4):
                k0 = 2 * b
                # cast X -> bf16 (DVE, one strided op per block)
                nc.vector.tensor_copy(
                    xb[:, k0 : k0 + 2, :D], xf[:, k0 : k0 + 2, :]
                )
                # X^2 -> bf16: ACT (from f32, independent of the cast) for
                # blocks 0-2; the last block on DVE right after its cast so
                # the tail isn't gated by ACT's serial queue
                if b < 3:
                    nc.scalar.activation(
                        out=xb[:, k0 : k0 + 2, D:],
                        in_=xf[:, k0 : k0 + 2, :],
                        func=mybir.ActivationFunctionType.Square,
                    )
                else:
                    nc.vector.tensor_mul(
                        xb[:, k0 : k0 + 2, D:],
                        xb[:, k0 : k0 + 2, :D],
                        xb[:, k0 : k0 + 2, :D],
                    )
                for k in (k0, k0 + 1):
                    lo = 0 if k % 2 == 0 else 64
                    nc.tensor.matmul(
                        acc[lo : lo + NCLS, :],
                        oh[:, k, :],
                        xb[:, k, :],
                        start=(k < 2),
                        stop=(k >= K - 2),
                        skip_group_check=True,
                    )

            # fold both chains' PSUM slices to SBUF (bf16) on two engines in
            # parallel, then one DMA covering partitions 0:104 (middle is
            # don't-care) so only a single issue+completion is paid
            nc.vector.tensor_copy(out_sb[:NCLS, :], acc[:NCLS, :])
            nc.scalar.copy(out_sb[64 : 64 + NCLS, :], acc[64 : 64 + NCLS, :])
            nc.sync.dma_start(out=stats[:, :], in_=out_sb[:OUTP, :])

    return nc


def _get_nc():
    if "nc" not in _CACHE:
        nc = _build_bass()
        nc.finalize()
        _CACHE["nc"] = nc
    return _CACHE["nc"]


_IOTA = np.broadcast_to(np.arange(NCLS, dtype=np.float32), (P, NCLS))


def run_device(output, classes, **spmd_kwargs):
    """Run the per-core Bass kernel; returns (list of per-core stats, results)."""
    x = np.ascontiguousarray(np.asarray(output), dtype=np.float32)
    cls_f = np.asarray(classes).astype(np.float32)
    in_maps = []
    for s in range(N_CORES):
        xs = x[s * ROWS : (s + 1) * ROWS].reshape(P, K, D)
        cs = cls_f[s * ROWS : (s + 1) * ROWS].reshape(P, K)
        combo = np.concatenate([_IOTA, cs], axis=1)
        in_maps.append(
            {"x": np.ascontiguousarray(xs), "combo": np.ascontiguousarray(combo)}
        )
    res = run_bass_kernel_spmd(
        _get_nc(), in_maps, core_ids=list(range(N_CORES)), **spmd_kwargs
    )
    stats = [res.results[s]["stats"] for s in range(N_CORES)]
    return stats, res


def _combine(stats, classes):
    """Combine per-core partial class stats into the scalar loss (float64)."""
    tot = np.sum(np.asarray(stats, dtype=np.float64), axis=0)  # [104, 512]
    tot = tot[:NCLS] + tot[64 : 64 + NCLS]                     # [40, 512]
    M_c = tot[:, :D]                                           # class sums
    SQ_c = tot[:, D:].sum(axis=1)                              # class |x|^2 sums
    n_c = np.bincount(np.asarray(classes).astype(np.int64), minlength=NCLS).astype(
        np.float64
    )
    SQ = SQ_c.sum()
    M = M_c.sum(axis=0)
    T_same = (2.0 * (n_c * SQ_c).sum() - 2.0 * (M_c * M_c).sum()) / D
    T_all = (2.0 * N * SQ - 2.0 * (M @ M)) / D
    loss = (2.0 * T_same - T_all) / (float(N) * float(N)) + BETA
    return np.float32(loss)


def kernel(output, classes):
    stats, _ = run_device(output, classes)
    return _combine(stats, classes)
